# revision 14
# baseline (speedup 1.0000x reference)
"""AdaptiveVectorQuantizerEMA (vq_codebook) — Trainium2 Bass kernel, 8 NeuronCores.

Data-parallel over the flattened token axis N = 64*2048 = 131072 (16384 rows
per core, 32 groups of 4 x 128-row subtiles).  Per subtile:
  PE   : transpose x (identity matmul), fp32 matmul s2m = x^T.T @ (2e^T)
  ACT  : square+accum -> xnorm; staging copies (walrus allows only ONE sync
         wait on a Matmult, so matmul inputs are routed through ACT)
  POOL : d = (enorm_rep + xnorm) - s2m   (scalar_tensor_tensor; replicates the
         reference's rounding shape fl(fl(xnorm+enorm) - fl(2m)), which keeps
         argmin tie-breaks consistent with the f32 reference)
  DVE  : prefix-min scan over d; first-argmin = #{k: prefix_min[k] > gmin};
         one-hot = (d == gmin) with a free duplicate-count accumulator
         (host zeroes dup rows and re-sets the single 1 at the device idx)
  DMA  : big contiguous loads/stores (host pre/post-shuffles layouts);
         SWDGE indirect gather e[idx] -> quantized.
Scalars (vq_loss, perplexity) are finished on the host from exact partials.

Layouts (G=4 subtiles/group, NG=rows/512 groups):
  inputs   [NG*128, 256] : x_dev[g*128+p, a*64+d] = x[g*512+a*128+p, d]
  enc      [NG*128, 2048]: enc_dev[g*128+p, a*512+k] = onehot[g*512+a*128+p, k]
  quant    [NG*128, 256] : like inputs
  idxcnt   [NG*128, 4]   : u32 idx + 1024*dupcount, col a = subtile a
  loss     [128, 1]      : per-partition sums of min distances
"""

import numpy as np

import concourse.bass as bass
import concourse.bacc as bacc
import concourse.mybir as mybir
from concourse.bass import IndirectOffsetOnAxis
from concourse.tile import TileContext

N_CORES = 8
N_FULL = 64 * 2048          # 131072
ROWS = N_FULL // N_CORES    # 16384 per core
K = 512
D = 64
G = 4                       # subtiles per group

f32 = mybir.dt.float32
u32 = mybir.dt.uint32
Alu = mybir.AluOpType
Act = mybir.ActivationFunctionType


def build_nc(rows=ROWS):
    ngroups = rows // (128 * G)
    nc = bacc.Bacc()
    x_in = nc.declare_dram_parameter("inputs", [ngroups * 128, G * D], f32,
                                     isOutput=False)
    e_in = nc.declare_dram_parameter("embedding", [K, D], f32, isOutput=False)
    h_in = nc.declare_dram_parameter("hconst", [128, 256], f32, isOutput=False)
    enc_out = nc.declare_dram_parameter("enc", [ngroups * 128, G * K], f32,
                                        isOutput=True)
    q_out = nc.declare_dram_parameter("quant", [ngroups * 128, G * D], f32,
                                      isOutput=True)
    ic_out = nc.declare_dram_parameter("idxcnt", [ngroups * 128, G], u32,
                                       isOutput=True)
    loss_out = nc.declare_dram_parameter("loss", [128, 1], f32, isOutput=True)

    with TileContext(nc) as tc:
        with (
            tc.tile_pool(name="const", bufs=1) as const,
            tc.tile_pool(name="xin", bufs=8) as xpool,
            tc.tile_pool(name="tps", bufs=2, space="PSUM") as tpsum,
            tc.tile_pool(name="sps", bufs=4, space="PSUM") as spsum,
            tc.tile_pool(name="wk", bufs=4) as work,
            tc.tile_pool(name="oh", bufs=2) as ohpool,
            tc.tile_pool(name="sm", bufs=4) as small,
        ):
            hconst = const.tile([128, 256], f32)
            eTr = const.tile([64, K], f32)       # 2 * e^T
            e2T = const.tile([64, K], f32)       # (e^T)^2 (unscaled)
            enorm_rep = const.tile([128, K], f32)
            loss_acc = const.tile([128, 1], f32)

            nc.gpsimd.memset(loss_acc[:], 0.0)

            # Host-provided constants (identity | ones), staged through ACT
            # so every matmul dependency collapses onto the ACT semaphore.
            hraw = work.tile([128, 256], f32, tag="hraw")
            nc.sync.dma_start(hraw[:], h_in[:, :])
            nc.scalar.copy(hconst[:], hraw[:])
            ident = hconst[:, 0:128]
            ones64 = hconst[0:64, 128:256]

            # e^T via PE transposes; chunks staged through ACT.
            for c in range(4):
                ec = work.tile([128, D], f32, tag="ec")
                nc.sync.dma_start(ec[:], e_in[c * 128:(c + 1) * 128, :])
                ecc = work.tile([128, D], f32, tag="ecc")
                nc.scalar.copy(ecc[:], ec[:])
                pt = tpsum.tile([64, 128], f32, tag="pt")
                nc.tensor.transpose(pt[:], ecc[:], ident)
                nc.scalar.activation(eTr[:, c * 128:(c + 1) * 128], pt[:],
                                     Act.Copy, scale=2.0)
            # (e^T)^2 = (2 e^T)^2 / 4 ; exact: x/2 and squaring are exact ops
            nc.scalar.activation(e2T[:], eTr[:], Act.Square, scale=0.5)
            # enorm_k replicated to all 128 partitions: ones64^T @ (eT*eT)
            ps_en = spsum.tile([128, K], f32, tag="sps")
            nc.tensor.matmul(out=ps_en[:], lhsT=ones64, rhs=e2T[:],
                             start=True, stop=True)
            nc.scalar.copy(enorm_rep[:], ps_en[:])

            for g in range(ngroups):
                p0 = g * 128
                xbig = xpool.tile([128, G * D], f32, tag="xbig")
                nc.sync.dma_start(xbig[:], x_in[p0:p0 + 128, :])

                onehbig = ohpool.tile([128, G * K], f32, tag="oneh")
                qtbig = work.tile([128, G * D], f32, tag="qtbig")
                idxf_g = small.tile([128, G], f32, tag="idxf")
                cntf_g = small.tile([128, G], f32, tag="cntf")

                for a in range(G):
                    sub = xbig[:, a * D:(a + 1) * D]
                    x2 = work.tile([128, D], f32, tag="x2")
                    xnorm = small.tile([128, 1], f32, tag="xn")
                    nc.scalar.activation(x2[:], sub, Act.Square,
                                         accum_out=xnorm[:])
                    xtc = work.tile([128, D], f32, tag="xtc")
                    nc.scalar.copy(xtc[:], sub)

                    pt = tpsum.tile([64, 128], f32, tag="pt")
                    nc.tensor.transpose(pt[:], xtc[:], ident)
                    xT = work.tile([64, 128], f32, tag="xT")
                    nc.scalar.copy(xT[:], pt[:])

                    ps = spsum.tile([128, K], f32, tag="sps")
                    nc.tensor.matmul(out=ps[:], lhsT=xT[:], rhs=eTr[:],
                                     start=True, stop=True)
                    s2m = work.tile([128, K], f32, tag="s2m")
                    nc.scalar.copy(s2m[:], ps[:])

                    # t_a = fl(enorm + xnorm) on POOL; d = fl(t_a - s2m) on
                    # DVE in 256-wide chunks (each chunk stays under the
                    # ~266ns pipe-drain threshold)
                    ta = work.tile([128, K], f32, tag="ta")
                    nc.gpsimd.tensor_scalar_add(ta[:], enorm_rep[:], xnorm[:])
                    dt = work.tile([128, K], f32, tag="d")
                    H = K // 2
                    for h in range(2):
                        sl = slice(h * H, (h + 1) * H)
                        nc.vector.tensor_tensor(
                            out=dt[:, sl], in0=ta[:, sl], in1=s2m[:, sl],
                            op=Alu.subtract)

                    # prefix-min along codes (chunk-chained); last = gmin
                    pmin = work.tile([128, K], f32, tag="pmin")
                    nc.vector.tensor_tensor_scan(
                        pmin[:, 0:H], dt[:, 0:H], dt[:, 0:H],
                        initial=3.0e38, op0=Alu.min, op1=Alu.bypass)
                    nc.vector.tensor_tensor_scan(
                        pmin[:, H:K], dt[:, H:K], dt[:, H:K],
                        initial=pmin[:, H - 1:H], op0=Alu.min, op1=Alu.bypass)
                    gmin = pmin[:, K - 1:K]

                    # first argmin = #( prefix_min > gmin )
                    scr = work.tile([128, K], f32, tag="scr")
                    nc.vector.tensor_scalar(
                        out=scr[:], in0=pmin[:], scalar1=gmin,
                        scalar2=None, op0=Alu.is_gt, op1=Alu.add,
                        accum_out=idxf_g[:, a:a + 1])

                    # one-hot (with possible dup on exact ties; count them)
                    nc.vector.tensor_scalar(
                        out=onehbig[:, a * K:(a + 1) * K], in0=dt[:],
                        scalar1=gmin, scalar2=None, op0=Alu.is_equal,
                        op1=Alu.add, accum_out=cntf_g[:, a:a + 1])

                    # loss partial: sum of per-row min distances
                    nc.vector.tensor_tensor(
                        out=loss_acc[:], in0=loss_acc[:], in1=gmin,
                        op=Alu.add)

                # gather offsets (u32 code indices) + packed idx/cnt output
                idxu_g = small.tile([128, G], u32, tag="idxu")
                nc.gpsimd.tensor_copy(idxu_g[:], idxf_g[:])
                comb = small.tile([128, G], f32, tag="comb")
                nc.gpsimd.tensor_scalar(
                    out=comb[:], in0=cntf_g[:], scalar1=1024.0, scalar2=None,
                    op0=Alu.mult)
                comb2 = small.tile([128, G], f32, tag="comb2")
                nc.gpsimd.tensor_tensor(
                    out=comb2[:], in0=comb[:], in1=idxf_g[:], op=Alu.add)
                icu = small.tile([128, G], u32, tag="icu")
                nc.gpsimd.tensor_copy(icu[:], comb2[:])

                # quantized rows: per-subtile gathers (HW indirect DMA uses
                # one offset per partition)
                for a in range(G):
                    nc.gpsimd.indirect_dma_start(
                        out=qtbig[:, a * D:(a + 1) * D], out_offset=None,
                        in_=e_in[:, :],
                        in_offset=IndirectOffsetOnAxis(
                            ap=idxu_g[:, a:a + 1], axis=0))

                nc.sync.dma_start(enc_out[p0:p0 + 128, :], onehbig[:])
                nc.sync.dma_start(q_out[p0:p0 + 128, :], qtbig[:])
                nc.sync.dma_start(ic_out[p0:p0 + 128, :], icu[:])

            nc.sync.dma_start(loss_out[:], loss_acc[:])
    nc.compile()
    return nc


_NC_CACHE = {}


def _get_nc(rows=ROWS):
    if rows not in _NC_CACHE:
        _NC_CACHE[rows] = build_nc(rows)
    return _NC_CACHE[rows]


TRACE = False
_LAST_RESULT = {}


def kernel(inputs, embedding):
    from concourse.bass_utils import run_bass_kernel_spmd

    x = np.ascontiguousarray(np.asarray(inputs, dtype=np.float32).reshape(-1, D))
    e = np.ascontiguousarray(np.asarray(embedding, dtype=np.float32))
    n = x.shape[0]
    rows = n // N_CORES
    ngroups = rows // (128 * G)
    nc = _get_nc(rows)

    hconst = np.concatenate(
        [np.eye(128, dtype=np.float32), np.ones((128, 128), np.float32)],
        axis=1)
    in_maps = []
    for c in range(N_CORES):
        xs = x[c * rows:(c + 1) * rows]
        # [NG,128,G*D]: x_dev[g, p, a*64+d] = xs[g*512 + a*128 + p, d]
        xdev = np.ascontiguousarray(
            xs.reshape(ngroups, G, 128, D).transpose(0, 2, 1, 3)
            .reshape(ngroups * 128, G * D))
        in_maps.append({"inputs": xdev, "embedding": e, "hconst": hconst})

    res = run_bass_kernel_spmd(nc, in_maps, core_ids=list(range(N_CORES)),
                               trace=TRACE)
    outs = res.results
    _LAST_RESULT["res"] = res

    enc_parts, q_parts, idx_parts, loss_sum = [], [], [], 0.0
    fix_rows_all = []
    base = 0
    for c in range(N_CORES):
        o = outs[c]
        enc_c = (o["enc"].reshape(ngroups, 128, G, K).transpose(0, 2, 1, 3)
                 .reshape(rows, K))
        q_c = (o["quant"].reshape(ngroups, 128, G, D).transpose(0, 2, 1, 3)
               .reshape(rows, D))
        ic_c = (o["idxcnt"].reshape(ngroups, 128, G).transpose(0, 2, 1)
                .reshape(rows))
        idx_c = (ic_c & 1023).astype(np.int32)
        cnt_c = (ic_c >> 10).astype(np.int32)
        bad = np.nonzero(cnt_c != 1)[0]
        if bad.size:
            enc_c[bad, :] = 0.0
            enc_c[bad, idx_c[bad]] = 1.0
            fix_rows_all.append(bad + base)
        enc_parts.append(enc_c)
        q_parts.append(q_c)
        idx_parts.append(idx_c)
        loss_sum += float(o["loss"].astype(np.float64).sum())
        base += rows

    enc = np.concatenate(enc_parts, axis=0)
    quant = np.concatenate(q_parts, axis=0).reshape(np.asarray(inputs).shape)
    idx = np.concatenate(idx_parts, axis=0)

    vq_loss = np.float32(1.25 * loss_sum / (n * D))
    counts = np.bincount(idx, minlength=K).astype(np.float64)
    p = counts / float(n)
    perplexity = np.float32(np.exp(-np.sum(p * np.log(p + 1e-10))))

    return vq_loss, quant, perplexity, enc, idx


# revision 18
# speedup vs baseline: 2.0295x; 2.0295x over previous
"""AdaptiveVectorQuantizerEMA (vq_codebook) — Trainium2 Bass kernel, 8 NeuronCores.

Data-parallel over the flattened token axis N = 64*2048 = 131072 (16384 rows
per core, 32 groups of 4 x 128-row subtiles).  Per subtile:
  PE   : transpose x (identity matmul), fp32 matmul s2m = x^T.T @ (2e^T)
  ACT  : square+accum -> xnorm; staging copies (walrus allows only ONE sync
         wait on a Matmult, so matmul inputs are routed through ACT)
  POOL : d = (enorm_rep + xnorm) - s2m   (scalar_tensor_tensor; replicates the
         reference's rounding shape fl(fl(xnorm+enorm) - fl(2m)), which keeps
         argmin tie-breaks consistent with the f32 reference)
  DVE  : prefix-min scan over d; first-argmin = #{k: prefix_min[k] > gmin};
         one-hot = (d == gmin) with a free duplicate-count accumulator
         (host zeroes dup rows and re-sets the single 1 at the device idx)
  DMA  : big contiguous loads/stores (host pre/post-shuffles layouts);
         SWDGE indirect gather e[idx] -> quantized.
Scalars (vq_loss, perplexity) are finished on the host from exact partials.

Layouts (G=4 subtiles/group, NG=rows/512 groups):
  inputs   [NG*128, 256] : x_dev[g*128+p, a*64+d] = x[g*512+a*128+p, d]
  enc      [NG*128, 2048]: enc_dev[g*128+p, a*512+k] = onehot[g*512+a*128+p, k]
  quant    [NG*128, 256] : like inputs
  idxcnt   [NG*128, 4]   : u32 idx + 1024*dupcount, col a = subtile a
  loss     [128, 1]      : per-partition sums of min distances
"""

import numpy as np

import concourse.bass as bass
import concourse.bacc as bacc
import concourse.mybir as mybir
from concourse.bass import IndirectOffsetOnAxis
from concourse.tile import TileContext

N_CORES = 8
N_FULL = 64 * 2048          # 131072
ROWS = N_FULL // N_CORES    # 16384 per core
K = 512
D = 64
G = 4                       # subtiles per group

f32 = mybir.dt.float32
u32 = mybir.dt.uint32
Alu = mybir.AluOpType
Act = mybir.ActivationFunctionType


def build_nc(rows=ROWS, no_scan=False, no_gather=False):
    ngroups = rows // (128 * G)
    nc = bacc.Bacc()
    x_in = nc.declare_dram_parameter("inputs", [ngroups * 128, G * D], f32,
                                     isOutput=False)
    e_in = nc.declare_dram_parameter("embedding", [K, D], f32, isOutput=False)
    h_in = nc.declare_dram_parameter("hconst", [128, 256], f32, isOutput=False)
    enc_out = nc.declare_dram_parameter("enc", [ngroups * 128, G * K], f32,
                                        isOutput=True)
    q_out = nc.declare_dram_parameter("quant", [ngroups * 128, G * D], f32,
                                      isOutput=True)
    ic_out = nc.declare_dram_parameter("idxcnt", [ngroups * 128, G], u32,
                                       isOutput=True)
    loss_out = nc.declare_dram_parameter("loss", [128, G], f32, isOutput=True)

    with TileContext(nc) as tc:
        with (
            tc.tile_pool(name="const", bufs=1) as const,
            tc.tile_pool(name="xin", bufs=8) as xpool,
            tc.tile_pool(name="tps", bufs=2, space="PSUM") as tpsum,
            tc.tile_pool(name="sps", bufs=4, space="PSUM") as spsum,
            tc.tile_pool(name="wk", bufs=6) as work,
            tc.tile_pool(name="oh", bufs=3) as ohpool,
            tc.tile_pool(name="sm", bufs=8) as small,
        ):
            hconst = const.tile([128, 256], f32)
            eTr = const.tile([64, K], f32)       # 2 * e^T
            e2T = const.tile([64, K], f32)       # (e^T)^2 (unscaled)
            enorm_rep = const.tile([128, K], f32)
            loss_acc = const.tile([128, G], f32)

            nc.gpsimd.memset(loss_acc[:], 0.0)

            # Host-provided constants (identity | ones), staged through ACT
            # so every matmul dependency collapses onto the ACT semaphore.
            hraw = work.tile([128, 256], f32, tag="hraw")
            nc.sync.dma_start(hraw[:], h_in[:, :])
            nc.scalar.copy(hconst[:], hraw[:])
            ident = hconst[:, 0:128]
            ones64 = hconst[0:64, 128:256]

            # e^T via PE transposes; chunks staged through ACT.
            for c in range(4):
                ec = work.tile([128, D], f32, tag="ec")
                nc.sync.dma_start(ec[:], e_in[c * 128:(c + 1) * 128, :])
                ecc = work.tile([128, D], f32, tag="ecc")
                nc.scalar.copy(ecc[:], ec[:])
                pt = tpsum.tile([64, 128], f32, tag="pt")
                nc.tensor.transpose(pt[:], ecc[:], ident)
                nc.scalar.activation(eTr[:, c * 128:(c + 1) * 128], pt[:],
                                     Act.Copy, scale=2.0)
            # (e^T)^2 = (2 e^T)^2 / 4 ; exact: x/2 and squaring are exact ops
            nc.scalar.activation(e2T[:], eTr[:], Act.Square, scale=0.5)
            # enorm_k replicated to all 128 partitions: ones64^T @ (eT*eT)
            ps_en = spsum.tile([128, K], f32, tag="sps")
            nc.tensor.matmul(out=ps_en[:], lhsT=ones64, rhs=e2T[:],
                             start=True, stop=True)
            nc.scalar.copy(enorm_rep[:], ps_en[:])

            for g in range(ngroups):
                p0 = g * 128
                xbig = xpool.tile([128, G * D], f32, tag="xbig")
                nc.sync.dma_start(xbig[:], x_in[p0:p0 + 128, :])

                onehbig = ohpool.tile([128, G * K], f32, tag="oneh")
                pmin_big = ohpool.tile([128, G * K], f32, tag="pminbig")
                qtbig = work.tile([128, G * D], f32, tag="qtbig")
                idxf_g = small.tile([128, G], f32, tag="idxf")
                cntf_g = small.tile([128, G], f32, tag="cntf")

                for a in range(G):
                    sub = xbig[:, a * D:(a + 1) * D]
                    x2 = work.tile([128, D], f32, tag="x2")
                    xnorm = small.tile([128, 1], f32, tag="xn")
                    nc.scalar.activation(x2[:], sub, Act.Square,
                                         accum_out=xnorm[:])
                    pt = tpsum.tile([64, 128], f32, tag="pt")
                    nc.tensor.transpose(pt[:], sub, ident)
                    xT = work.tile([64, 128], f32, tag="xT")
                    nc.scalar.copy(xT[:], pt[:])

                    ps = spsum.tile([128, K], f32, tag="sps")
                    nc.tensor.matmul(out=ps[:], lhsT=xT[:], rhs=eTr[:],
                                     start=True, stop=True)
                    s2m = ps

                    # t_a = fl(enorm + xnorm): ACT Identity with per-partition
                    # bias reproduces the reference's single-rounding add
                    ta = work.tile([128, K], f32, tag="ta")
                    nc.scalar.activation(ta[:], enorm_rep[:], Act.Identity,
                                         bias=xnorm[:], scale=1.0)
                    dt = work.tile([128, K], f32, tag="d")
                    nc.vector.tensor_tensor(
                        out=dt[:], in0=ta[:], in1=s2m[:], op=Alu.subtract)

                    # prefix-min along codes; last element = gmin
                    pmin = pmin_big[:, a * K:(a + 1) * K]
                    if no_scan:
                        nc.vector.tensor_tensor(
                            out=pmin, in0=dt[:], in1=dt[:], op=Alu.min)
                    else:
                        nc.vector.tensor_tensor_scan(
                            pmin, dt[:], dt[:],
                            initial=3.0e38, op0=Alu.min, op1=Alu.bypass)
                    gmin = pmin_big[:, a * K + K - 1:a * K + K]

                    # first argmin = #( prefix_min > gmin )
                    scr = work.tile([128, K], f32, tag="scr")
                    nc.vector.tensor_scalar(
                        out=scr[:], in0=pmin, scalar1=gmin,
                        scalar2=None, op0=Alu.is_gt, op1=Alu.add,
                        accum_out=idxf_g[:, a:a + 1])

                    # one-hot (with possible dup on exact ties; count them)
                    nc.vector.tensor_scalar(
                        out=onehbig[:, a * K:(a + 1) * K], in0=dt[:],
                        scalar1=gmin, scalar2=None, op0=Alu.is_equal,
                        op1=Alu.add, accum_out=cntf_g[:, a:a + 1])

                # loss partial: add the 4 per-subtile minima in one op
                gmin4 = pmin_big[:, K - 1::K]
                nc.vector.tensor_tensor(
                    out=loss_acc[:], in0=loss_acc[:], in1=gmin4, op=Alu.add)

                # gather offsets (u32 code indices) + packed idx/cnt output
                # (small casts on DVE; GPSIMD Q7 compute is ~10x slower)
                idxu_g = small.tile([128, G], u32, tag="idxu")
                nc.vector.tensor_copy(idxu_g[:], idxf_g[:])
                comb = small.tile([128, G], f32, tag="comb")
                nc.vector.tensor_scalar(
                    out=comb[:], in0=cntf_g[:], scalar1=1024.0, scalar2=None,
                    op0=Alu.mult)
                comb2 = small.tile([128, G], f32, tag="comb2")
                nc.vector.tensor_tensor(
                    out=comb2[:], in0=comb[:], in1=idxf_g[:], op=Alu.add)
                icu = small.tile([128, G], u32, tag="icu")
                nc.vector.tensor_copy(icu[:], comb2[:])

                # quantized rows: per-subtile gathers (HW indirect DMA uses
                # one offset per partition)
                if not no_gather:
                    for a in range(G):
                        nc.gpsimd.indirect_dma_start(
                            out=qtbig[:, a * D:(a + 1) * D], out_offset=None,
                            in_=e_in[:, :],
                            in_offset=IndirectOffsetOnAxis(
                                ap=idxu_g[:, a:a + 1], axis=0))

                nc.sync.dma_start(enc_out[p0:p0 + 128, :], onehbig[:])
                nc.sync.dma_start(q_out[p0:p0 + 128, :],
                                  xbig[:] if no_gather else qtbig[:])
                nc.sync.dma_start(ic_out[p0:p0 + 128, :], icu[:])

            nc.sync.dma_start(loss_out[:], loss_acc[:])
    nc.compile()
    return nc


_NC_CACHE = {}


def _get_nc(rows=ROWS):
    if rows not in _NC_CACHE:
        _NC_CACHE[rows] = build_nc(rows)
    return _NC_CACHE[rows]


TRACE = False
_LAST_RESULT = {}


def kernel(inputs, embedding):
    from concourse.bass_utils import run_bass_kernel_spmd

    x = np.ascontiguousarray(np.asarray(inputs, dtype=np.float32).reshape(-1, D))
    e = np.ascontiguousarray(np.asarray(embedding, dtype=np.float32))
    n = x.shape[0]
    rows = n // N_CORES
    ngroups = rows // (128 * G)
    nc = _get_nc(rows)

    hconst = np.concatenate(
        [np.eye(128, dtype=np.float32), np.ones((128, 128), np.float32)],
        axis=1)
    in_maps = []
    for c in range(N_CORES):
        xs = x[c * rows:(c + 1) * rows]
        # [NG,128,G*D]: x_dev[g, p, a*64+d] = xs[g*512 + a*128 + p, d]
        xdev = np.ascontiguousarray(
            xs.reshape(ngroups, G, 128, D).transpose(0, 2, 1, 3)
            .reshape(ngroups * 128, G * D))
        in_maps.append({"inputs": xdev, "embedding": e, "hconst": hconst})

    res = run_bass_kernel_spmd(nc, in_maps, core_ids=list(range(N_CORES)),
                               trace=TRACE)
    outs = res.results
    _LAST_RESULT["res"] = res

    enc_parts, q_parts, idx_parts, loss_sum = [], [], [], 0.0
    fix_rows_all = []
    base = 0
    for c in range(N_CORES):
        o = outs[c]
        enc_c = (o["enc"].reshape(ngroups, 128, G, K).transpose(0, 2, 1, 3)
                 .reshape(rows, K))
        q_c = (o["quant"].reshape(ngroups, 128, G, D).transpose(0, 2, 1, 3)
               .reshape(rows, D))
        ic_c = (o["idxcnt"].reshape(ngroups, 128, G).transpose(0, 2, 1)
                .reshape(rows))
        idx_c = (ic_c & 1023).astype(np.int32)
        cnt_c = (ic_c >> 10).astype(np.int32)
        bad = np.nonzero(cnt_c != 1)[0]
        if bad.size:
            enc_c[bad, :] = 0.0
            enc_c[bad, idx_c[bad]] = 1.0
            fix_rows_all.append(bad + base)
        enc_parts.append(enc_c)
        q_parts.append(q_c)
        idx_parts.append(idx_c)
        loss_sum += float(o["loss"].astype(np.float64).sum())
        base += rows

    enc = np.concatenate(enc_parts, axis=0)
    quant = np.concatenate(q_parts, axis=0).reshape(np.asarray(inputs).shape)
    idx = np.concatenate(idx_parts, axis=0)

    vq_loss = np.float32(1.25 * loss_sum / (n * D))
    counts = np.bincount(idx, minlength=K).astype(np.float64)
    p = counts / float(n)
    perplexity = np.float32(np.exp(-np.sum(p * np.log(p + 1e-10))))

    return vq_loss, quant, perplexity, enc, idx
